# revision 28
# baseline (speedup 1.0000x reference)
"""Trainium2 Bass kernel for nn_CustomConv2d: 3x3 conv, B=16, Cin=Cout=128, H=W=64.

Strategy (v5):
  - Data-parallel over batch: 8 NeuronCores x 2 images each; the (128,128,9)
    weight is replicated (host pre-transposes to [cin, tap, cout] so tap k is
    a contiguous [cin, cout] stationary-operand slice).
  - fp16 matmuls (1 cycle/row like fp32r, 10-bit mantissa ~ fp32r precision,
    half the DMA bytes and half the LDWEIGHTS time; PSUM accumulates fp32).
  - Per image the feature map lives in SBUF as a 66x66 zero-padded plane
    (host-prepadded => every DMA is contiguous per partition).
  - Conv = 9 accumulating PE matmuls per 8-row output block (contraction over
    Cin=128 on the partition dim).
  - DMA plan: the two HWDGE engines (sync=SP, scalar=Activation) each own a
    hardware ring; the 16 DMA queues round-robin across rings, so bandwidth
    splits evenly between rings whenever both are loaded.  All input DMAs are
    therefore issued in strict consumption order, alternating sync/scalar, so
    the earliest-needed bytes always lead both rings; output DMAs follow.
  - PE warm-up: HAM un-throttles the PE clock only after ~3us of sustained
    array activity and re-throttles after idle windows, so fp16 junk matmuls
    (zero tile memset on the otherwise-idle vector engine) bridge from engine
    start (~6.5us) to first-conv-data (~10us) with no PE gap.
  - Tail: final block in halves, copies on vector+scalar in parallel, DMAs on
    sync+scalar, so the exit drain starts as soon as possible.
"""

import numpy as np

import concourse.bass as bass  # noqa: F401  (registers bass types)
import concourse.tile as tile
import concourse.mybir as mybir
from concourse import bacc, bass_utils

F32 = mybir.dt.float32
F16 = mybir.dt.float16

B, CIN, COUT, KK, H, W = 16, 128, 128, 3, 64, 64
NCORES = 8
BPC = B // NCORES  # images per core
HW = H * W         # 4096
PW = W + 2         # padded row length (66)
PH = H + 2         # padded rows (66)
XLEN = PH * PW     # 4356
ROWBLK = 8         # output rows per PSUM block (8*64=512 = one fp32 PSUM bank)
NBLK = H // ROWBLK # 8 blocks per image

WARMN = 9          # warmup matmuls (bridge engine-start -> first data ready)
TRACE = False      # set True to capture an NTFF profile (fills LAST_EXEC_NS)
LAST_EXEC_NS = None

_CACHE = {}

# img0 x chunks (padded-row ranges), consumption-ordered; block yb needs rows
# [8yb, 8yb+10).  The first conv matmul (tap dy=0) needs only rows 0-7, and
# consumer visibility = chunk ring-completion + ~2.4us semaphore latency, so
# the first chunk is exactly rows 0-7.  img1 in 3 coarser chunks (~28us+).
CHUNKS0 = [(0, 8), (8, 18), (18, 34), (34, 50), (50, PH)]
CHUNKS1 = [(0, 22), (22, 44), (44, PH)]


def _build():
    nc = bacc.Bacc("TRN2", target_bir_lowering=False, debug=False, num_devices=NCORES)
    x_d = nc.dram_tensor("x", [BPC, CIN, XLEN], F16, kind="ExternalInput").ap()
    w_d = nc.dram_tensor("w", [CIN, KK * KK * COUT], F16, kind="ExternalInput").ap()
    o_d = nc.dram_tensor("o", [BPC, COUT, HW], F32, kind="ExternalOutput").ap()

    with tile.TileContext(nc) as tc:
        with (
            tc.tile_pool(name="wt", bufs=1) as wtp,
            tc.tile_pool(name="xin", bufs=2) as xp,
            tc.tile_pool(name="ps", bufs=6, space="PSUM") as pp,
            tc.tile_pool(name="ot", bufs=4) as op,
            tc.tile_pool(name="warm", bufs=1) as wmp,
            tc.tile_pool(name="warmps", bufs=1, space="PSUM") as wpp,
        ):
            # --- warmup: keep the PE busy from engine start until data lands.
            wz = wmp.tile([CIN, 4 * COUT], F16)
            nc.vector.memset(wz[:], 0.0)
            wps = wpp.tile([COUT, 4 * COUT], F32)
            for _ in range(WARMN):
                nc.tensor.matmul(wps[:], wz[:, :COUT], wz[:], start=True, stop=True)

            # --- input DMAs.  The two HWDGE engines (sync, scalar) each own a
            # ring; descriptor generation is the early bottleneck (~1us per
            # 128-descriptor DMA, serialized per ring).  Tile tracks the wt
            # tile's readers against its LAST write, so all 3 w-group DMAs go
            # first (the first conv matmul gates on w678's semaphore).
            wt = wtp.tile([CIN, KK * KK * COUT], F16)
            xins = []
            for lb in range(BPC):
                xin = xp.tile([CIN, XLEN], F16, tag="xin")
                xins.append(xin)

            def xc(lb, r0, r1):
                return (xins[lb][:, PW * r0 : PW * r1], x_d[lb][:, PW * r0 : PW * r1])

            wg = [
                (wt[:, g * 3 * COUT : (g + 1) * 3 * COUT],
                 w_d[:, g * 3 * COUT : (g + 1) * 3 * COUT])
                for g in range(3)
            ]
            ring = [nc.sync, nc.scalar]
            c = [xc(0, r0, r1) for r0, r1 in CHUNKS0]
            i1 = [xc(1, r0, r1) for r0, r1 in CHUNKS1]
            # sync's ring starts ~0.8us earlier; it carries w taps 0-2 and the
            # first-matmul rows.  Everything ordered by consumption deadline;
            # each DMA costs ~0.5us of fixed ring pacing, so chunks are merged
            # up to the granularity their deadline allows.
            sync_plan = [wg[0], c[0], c[1], c[3], c[4], i1[1]]
            scalar_plan = [wg[1], wg[2], c[2], i1[0], i1[2]]
            for eng, plan in ((nc.sync, sync_plan), (nc.scalar, scalar_plan)):
                for dst, src in plan:
                    eng.dma_start(dst, src)

            # --- conv: 9 accumulating matmuls per 8-row block ---
            nout = 0
            for lb in range(BPC):
                xrf = xins[lb][:].rearrange("p (r c) -> p r c", c=PW)  # [128,66,66]
                for yb in range(NBLK):
                    y0 = yb * ROWBLK
                    ps = pp.tile([COUT, ROWBLK * W], F32)
                    first = True
                    for dy in range(KK):
                        for dx in range(KK):
                            t = dy * KK + dx
                            nc.tensor.matmul(
                                ps[:],
                                wt[:, t * COUT : (t + 1) * COUT],
                                xrf[:, y0 + dy : y0 + dy + ROWBLK, dx : dx + W],
                                start=first,
                                stop=(dy == KK - 1 and dx == KK - 1),
                            )
                            first = False
                    ot = op.tile([COUT, ROWBLK * W], F32)
                    if lb == BPC - 1 and yb == NBLK - 1:
                        # final block split by PARTITION (cout halves): copies
                        # run on vector+scalar in parallel and each output DMA
                        # only needs 64 descriptors, halving the exit chain.
                        ph = COUT // 2
                        for h_, deng in ((0, nc.sync), (1, nc.scalar)):
                            sl = slice(h_ * ph, (h_ + 1) * ph)
                            if h_ == 0:
                                nc.vector.tensor_copy(ot[sl, :], ps[sl, :])
                            else:
                                nc.scalar.copy(ot[sl, :], ps[sl, :])
                            deng.dma_start(
                                o_d[lb][sl, W * y0 : W * y0 + ROWBLK * W],
                                ot[sl, :],
                            )
                    else:
                        nc.vector.tensor_copy(ot[:], ps[:])
                        # alternate output rings too (inputs are all queued
                        # ahead of every output on both engines)
                        ring[nout % 2].dma_start(
                            o_d[lb][:, W * y0 : W * y0 + ROWBLK * W], ot[:]
                        )
                        nout += 1
    nc.compile()
    return nc


def _get_nc():
    key = ("nc_v9", WARMN)
    if key not in _CACHE:
        _CACHE[key] = _build()
    return _CACHE[key]


def kernel(x, weights):
    """x: [16,128,64,64] f32; weights: [128,128,9] f32 -> [2048,64,64] f32."""
    global LAST_EXEC_NS
    x = np.asarray(x, dtype=np.float32)
    w = np.asarray(weights, dtype=np.float32)
    # [cout, cin, k] -> [cin, k, cout] so tap k is a contiguous lhsT slice
    wT = np.ascontiguousarray(w.transpose(1, 2, 0)).reshape(CIN, KK * KK * COUT)
    wT = wT.astype(np.float16)
    xpad = np.zeros((B, CIN, PH, PW), np.float16)
    xpad[:, :, 1 : H + 1, 1 : W + 1] = x.astype(np.float16)
    xpad = xpad.reshape(B, CIN, XLEN)

    nc = _get_nc()
    xr = xpad.reshape(NCORES, BPC, CIN, XLEN)
    in_maps = [{"x": np.ascontiguousarray(xr[c]), "w": wT} for c in range(NCORES)]

    res = bass_utils.run_bass_kernel_spmd(
        nc, in_maps, core_ids=list(range(NCORES)), trace=TRACE
    )
    LAST_EXEC_NS = res.exec_time_ns

    arr = np.stack([res.results[c]["o"] for c in range(NCORES)])  # [8, 2, 128, 4096]
    # out[cout*B + b] = conv[b, cout], with b = core*BPC + lb
    arr = arr.transpose(2, 0, 1, 3).reshape(COUT, B, H, W)
    return np.ascontiguousarray(arr.reshape(COUT * B, H, W))


# revision 31
# speedup vs baseline: 1.0041x; 1.0041x over previous
"""Trainium2 Bass kernel for nn_CustomConv2d: 3x3 conv, B=16, Cin=Cout=128, H=W=64.

Strategy (v5):
  - Data-parallel over batch: 8 NeuronCores x 2 images each; the (128,128,9)
    weight is replicated (host pre-transposes to [cin, tap, cout] so tap k is
    a contiguous [cin, cout] stationary-operand slice).
  - fp16 matmuls (1 cycle/row like fp32r, 10-bit mantissa ~ fp32r precision,
    half the DMA bytes and half the LDWEIGHTS time; PSUM accumulates fp32).
  - Per image the feature map lives in SBUF as a 66x66 zero-padded plane
    (host-prepadded => every DMA is contiguous per partition).
  - Conv = 9 accumulating PE matmuls per 8-row output block (contraction over
    Cin=128 on the partition dim).
  - DMA plan: the two HWDGE engines (sync=SP, scalar=Activation) each own a
    hardware ring; the 16 DMA queues round-robin across rings, so bandwidth
    splits evenly between rings whenever both are loaded.  All input DMAs are
    therefore issued in strict consumption order, alternating sync/scalar, so
    the earliest-needed bytes always lead both rings; output DMAs follow.
  - PE warm-up: HAM un-throttles the PE clock only after ~3us of sustained
    array activity and re-throttles after idle windows, so fp16 junk matmuls
    (zero tile memset on the otherwise-idle vector engine) bridge from engine
    start (~6.5us) to first-conv-data (~10us) with no PE gap.
  - Tail: final block in halves, copies on vector+scalar in parallel, DMAs on
    sync+scalar, so the exit drain starts as soon as possible.
"""

import numpy as np

import concourse.bass as bass  # noqa: F401  (registers bass types)
import concourse.tile as tile
import concourse.mybir as mybir
from concourse import bacc, bass_utils

F32 = mybir.dt.float32
F16 = mybir.dt.float16

B, CIN, COUT, KK, H, W = 16, 128, 128, 3, 64, 64
NCORES = 8
BPC = B // NCORES  # images per core
HW = H * W         # 4096
PW = W + 2         # padded row length (66)
PH = H + 2         # padded rows (66)
XLEN = PH * PW     # 4356
ROWBLK = 8         # output rows per PSUM block (8*64=512 = one fp32 PSUM bank)
NBLK = H // ROWBLK # 8 blocks per image

WARMN = 9          # warmup matmuls (bridge engine-start -> first data ready)
TRACE = False      # set True to capture an NTFF profile (fills LAST_EXEC_NS)
LAST_EXEC_NS = None

_CACHE = {}

# img0 x chunks (padded-row ranges), consumption-ordered; block yb needs rows
# [8yb, 8yb+10).  The first conv matmul (tap dy=0) needs only rows 0-7, and
# consumer visibility = chunk ring-completion + ~2.4us semaphore latency, so
# the first chunk is exactly rows 0-7.  img1 in 3 coarser chunks (~28us+).
CHUNKS0 = [(0, 8), (8, 18), (18, 34), (34, 50), (50, PH)]
CHUNKS1 = [(0, 22), (22, 44), (44, PH)]


def _build():
    nc = bacc.Bacc("TRN2", target_bir_lowering=False, debug=False, num_devices=NCORES)
    x_d = nc.dram_tensor("x", [BPC, CIN, XLEN], F16, kind="ExternalInput").ap()
    w_d = nc.dram_tensor("w", [CIN, KK * KK * COUT], F16, kind="ExternalInput").ap()
    o_d = nc.dram_tensor("o", [BPC, COUT, HW], F32, kind="ExternalOutput").ap()

    with tile.TileContext(nc) as tc:
        with (
            tc.tile_pool(name="wt", bufs=1) as wtp,
            tc.tile_pool(name="xin", bufs=2) as xp,
            tc.tile_pool(name="ps", bufs=6, space="PSUM") as pp,
            tc.tile_pool(name="ot", bufs=4) as op,
            tc.tile_pool(name="warm", bufs=1) as wmp,
            tc.tile_pool(name="warmps", bufs=1, space="PSUM") as wpp,
        ):
            # --- warmup: keep the PE busy from engine start until data lands.
            wz = wmp.tile([CIN, 4 * COUT], F16)
            nc.vector.memset(wz[:], 0.0)
            wps = wpp.tile([COUT, 4 * COUT], F32)
            for _ in range(WARMN):
                nc.tensor.matmul(wps[:], wz[:, :COUT], wz[:], start=True, stop=True)

            # --- input DMAs.  The two HWDGE engines (sync, scalar) each own a
            # ring; descriptor generation is the early bottleneck (~1us per
            # 128-descriptor DMA, serialized per ring).  Tile tracks the wt
            # tile's readers against its LAST write, so all 3 w-group DMAs go
            # first (the first conv matmul gates on w678's semaphore).
            wt = wtp.tile([CIN, KK * KK * COUT], F16)
            xins = []
            for lb in range(BPC):
                xin = xp.tile([CIN, XLEN], F16, tag="xin")
                xins.append(xin)

            def xc(lb, r0, r1):
                return (xins[lb][:, PW * r0 : PW * r1], x_d[lb][:, PW * r0 : PW * r1])

            wg = [
                (wt[:, g * 3 * COUT : (g + 1) * 3 * COUT],
                 w_d[:, g * 3 * COUT : (g + 1) * 3 * COUT])
                for g in range(3)
            ]
            ring = [nc.sync, nc.scalar]
            c = [xc(0, r0, r1) for r0, r1 in CHUNKS0]
            i1 = [xc(1, r0, r1) for r0, r1 in CHUNKS1]
            # sync's ring starts ~0.8us earlier; it carries w taps 0-2 and the
            # first-matmul rows.  Everything ordered by consumption deadline;
            # each DMA costs ~0.5us of fixed ring pacing, so chunks are merged
            # up to the granularity their deadline allows.
            sync_plan = [wg[0], c[0], c[1], c[3], c[4], i1[1]]
            scalar_plan = [wg[1], wg[2], c[2], i1[0], i1[2]]
            for eng, plan in ((nc.sync, sync_plan), (nc.scalar, scalar_plan)):
                for dst, src in plan:
                    eng.dma_start(dst, src)

            # --- conv: 9 accumulating matmuls per 8-row block ---
            nout = 0
            for lb in range(BPC):
                xrf = xins[lb][:].rearrange("p (r c) -> p r c", c=PW)  # [128,66,66]
                for yb in range(NBLK):
                    y0 = yb * ROWBLK
                    ps = pp.tile([COUT, ROWBLK * W], F32)
                    first = True
                    for dy in range(KK):
                        for dx in range(KK):
                            t = dy * KK + dx
                            nc.tensor.matmul(
                                ps[:],
                                wt[:, t * COUT : (t + 1) * COUT],
                                xrf[:, y0 + dy : y0 + dy + ROWBLK, dx : dx + W],
                                start=first,
                                stop=(dy == KK - 1 and dx == KK - 1),
                            )
                            first = False
                    if lb == BPC - 1 and yb == NBLK - 1:
                        ot = op.tile([COUT, ROWBLK * W], F32, tag="otf")
                        # final block split by PARTITION (cout halves): copies
                        # run on vector+scalar in parallel and each output DMA
                        # only needs 64 descriptors, halving the exit chain.
                        ph = COUT // 2
                        for h_, deng in ((0, nc.sync), (1, nc.scalar)):
                            sl = slice(h_ * ph, (h_ + 1) * ph)
                            if h_ == 0:
                                nc.vector.tensor_copy(ot[sl, :], ps[sl, :])
                            else:
                                nc.scalar.copy(ot[sl, :], ps[sl, :])
                            deng.dma_start(
                                o_d[lb][sl, W * y0 : W * y0 + ROWBLK * W],
                                ot[sl, :],
                            )
                    elif lb == BPC - 1 and yb == NBLK - 2:
                        # block before the final one ships alone (its pair
                        # partner takes the exit-critical path)
                        ot = op.tile([COUT, ROWBLK * W], F32, tag="ot1")
                        nc.vector.tensor_copy(ot[:], ps[:])
                        ring[nout % 2].dma_start(
                            o_d[lb][:, W * y0 : W * y0 + ROWBLK * W], ot[:]
                        )
                        nout += 1
                    else:
                        # stage two blocks per SBUF tile; ship them as one
                        # 2-block DMA (fewer DMAs = less ring pacing overhead
                        # ahead of the exit-critical final DMA)
                        if yb % 2 == 0:
                            otp = op.tile([COUT, 2 * ROWBLK * W], F32, tag="otp")
                        nc.vector.tensor_copy(
                            otp[:, (yb % 2) * ROWBLK * W : (yb % 2 + 1) * ROWBLK * W],
                            ps[:],
                        )
                        if yb % 2 == 1:
                            ring[nout % 2].dma_start(
                                o_d[lb][:, W * (y0 - ROWBLK) : W * y0 + ROWBLK * W],
                                otp[:],
                            )
                            nout += 1
    nc.compile()
    return nc


def _get_nc():
    key = ("nc_v10", WARMN)
    if key not in _CACHE:
        _CACHE[key] = _build()
    return _CACHE[key]


def kernel(x, weights):
    """x: [16,128,64,64] f32; weights: [128,128,9] f32 -> [2048,64,64] f32."""
    global LAST_EXEC_NS
    x = np.asarray(x, dtype=np.float32)
    w = np.asarray(weights, dtype=np.float32)
    # [cout, cin, k] -> [cin, k, cout] so tap k is a contiguous lhsT slice
    wT = np.ascontiguousarray(w.transpose(1, 2, 0)).reshape(CIN, KK * KK * COUT)
    wT = wT.astype(np.float16)
    xpad = np.zeros((B, CIN, PH, PW), np.float16)
    xpad[:, :, 1 : H + 1, 1 : W + 1] = x.astype(np.float16)
    xpad = xpad.reshape(B, CIN, XLEN)

    nc = _get_nc()
    xr = xpad.reshape(NCORES, BPC, CIN, XLEN)
    in_maps = [{"x": np.ascontiguousarray(xr[c]), "w": wT} for c in range(NCORES)]

    res = bass_utils.run_bass_kernel_spmd(
        nc, in_maps, core_ids=list(range(NCORES)), trace=TRACE
    )
    LAST_EXEC_NS = res.exec_time_ns

    arr = np.stack([res.results[c]["o"] for c in range(NCORES)])  # [8, 2, 128, 4096]
    # out[cout*B + b] = conv[b, cout], with b = core*BPC + lb
    arr = arr.transpose(2, 0, 1, 3).reshape(COUT, B, H, W)
    return np.ascontiguousarray(arr.reshape(COUT * B, H, W))


# revision 34
# speedup vs baseline: 1.0204x; 1.0163x over previous
"""Trainium2 Bass kernel for nn_CustomConv2d: 3x3 conv, B=16, Cin=Cout=128, H=W=64.

Strategy (v5):
  - Data-parallel over batch: 8 NeuronCores x 2 images each; the (128,128,9)
    weight is replicated (host pre-transposes to [cin, tap, cout] so tap k is
    a contiguous [cin, cout] stationary-operand slice).
  - fp16 matmuls (1 cycle/row like fp32r, 10-bit mantissa ~ fp32r precision,
    half the DMA bytes and half the LDWEIGHTS time; PSUM accumulates fp32).
  - Per image the feature map lives in SBUF as a 66x66 zero-padded plane
    (host-prepadded => every DMA is contiguous per partition).
  - Conv = 9 accumulating PE matmuls per 8-row output block (contraction over
    Cin=128 on the partition dim).
  - DMA plan: the two HWDGE engines (sync=SP, scalar=Activation) each own a
    hardware ring; the 16 DMA queues round-robin across rings, so bandwidth
    splits evenly between rings whenever both are loaded.  All input DMAs are
    therefore issued in strict consumption order, alternating sync/scalar, so
    the earliest-needed bytes always lead both rings; output DMAs follow.
  - PE warm-up: HAM un-throttles the PE clock only after ~3us of sustained
    array activity and re-throttles after idle windows, so fp16 junk matmuls
    (zero tile memset on the otherwise-idle vector engine) bridge from engine
    start (~6.5us) to first-conv-data (~10us) with no PE gap.
  - Tail: final block in halves, copies on vector+scalar in parallel, DMAs on
    sync+scalar, so the exit drain starts as soon as possible.
"""

import numpy as np

import concourse.bass as bass  # noqa: F401  (registers bass types)
import concourse.tile as tile
import concourse.mybir as mybir
from concourse import bacc, bass_utils

F32 = mybir.dt.float32
F16 = mybir.dt.float16

B, CIN, COUT, KK, H, W = 16, 128, 128, 3, 64, 64
NCORES = 8
BPC = B // NCORES  # images per core
HW = H * W         # 4096
PW = W + 2         # padded row length (66)
PH = H + 2         # padded rows (66)
XLEN = PH * PW     # 4356
ROWBLK = 8         # output rows per PSUM block (8*64=512 = one fp32 PSUM bank)
NBLK = H // ROWBLK # 8 blocks per image

WARMN = 8          # warmup matmuls (bridge engine-start -> first data ready)
TRACE = False      # set True to capture an NTFF profile (fills LAST_EXEC_NS)
LAST_EXEC_NS = None

_CACHE = {}

# img0 x chunks (padded-row ranges), consumption-ordered; block yb needs rows
# [8yb, 8yb+10).  The first conv matmul (tap dy=0) needs only rows 0-7, and
# consumer visibility = chunk ring-completion + ~2.4us semaphore latency, so
# the first chunk is exactly rows 0-7.  img1 in 3 coarser chunks (~28us+).
CHUNKS0 = [(0, 8), (8, 18), (18, 34), (34, 50), (50, PH)]
CHUNKS1 = [(0, 22), (22, 44), (44, PH)]


def _build():
    nc = bacc.Bacc("TRN2", target_bir_lowering=False, debug=False, num_devices=NCORES)
    x_d = nc.dram_tensor("x", [BPC, CIN, XLEN], F16, kind="ExternalInput").ap()
    w_d = nc.dram_tensor("w", [CIN, KK * KK * COUT], F16, kind="ExternalInput").ap()
    o_d = nc.dram_tensor("o", [BPC, COUT, HW], F32, kind="ExternalOutput").ap()

    with tile.TileContext(nc) as tc:
        with (
            tc.tile_pool(name="wt", bufs=1) as wtp,
            tc.tile_pool(name="xin", bufs=2) as xp,
            tc.tile_pool(name="ps", bufs=6, space="PSUM") as pp,
            tc.tile_pool(name="ot", bufs=4) as op,
            tc.tile_pool(name="warm", bufs=1) as wmp,
            tc.tile_pool(name="warmps", bufs=1, space="PSUM") as wpp,
        ):
            # --- warmup: keep the PE busy from engine start until data lands.
            wz = wmp.tile([CIN, 4 * COUT], F16)
            nc.vector.memset(wz[:], 0.0)
            wps = wpp.tile([COUT, 4 * COUT], F32)
            for _ in range(WARMN):
                nc.tensor.matmul(wps[:], wz[:, :COUT], wz[:], start=True, stop=True)

            # --- input DMAs.  The two HWDGE engines (sync, scalar) each own a
            # ring; descriptor generation is the early bottleneck (~1us per
            # 128-descriptor DMA, serialized per ring).  Tile tracks the wt
            # tile's readers against its LAST write, so all 3 w-group DMAs go
            # first (the first conv matmul gates on w678's semaphore).
            wt = wtp.tile([CIN, KK * KK * COUT], F16)
            xins = []
            for lb in range(BPC):
                xin = xp.tile([CIN, XLEN], F16, tag="xin")
                xins.append(xin)

            def xc(lb, r0, r1):
                return (xins[lb][:, PW * r0 : PW * r1], x_d[lb][:, PW * r0 : PW * r1])

            wg = [
                (wt[:, g * 3 * COUT : (g + 1) * 3 * COUT],
                 w_d[:, g * 3 * COUT : (g + 1) * 3 * COUT])
                for g in range(3)
            ]
            ring = [nc.sync, nc.scalar]
            c = [xc(0, r0, r1) for r0, r1 in CHUNKS0]
            i1 = [xc(1, r0, r1) for r0, r1 in CHUNKS1]
            # sync's ring starts ~0.8us earlier; it carries w taps 0-2 and the
            # first-matmul rows.  Everything ordered by consumption deadline;
            # each DMA costs ~0.5us of fixed ring pacing, so chunks are merged
            # up to the granularity their deadline allows.
            sync_plan = [wg[0], c[0], c[1], c[3], c[4], i1[1]]
            scalar_plan = [wg[1], wg[2], c[2], i1[0], i1[2]]
            for eng, plan in ((nc.sync, sync_plan), (nc.scalar, scalar_plan)):
                for dst, src in plan:
                    eng.dma_start(dst, src)

            # --- conv: 9 accumulating matmuls per 8-row block ---
            nout = 0
            for lb in range(BPC):
                xrf = xins[lb][:].rearrange("p (r c) -> p r c", c=PW)  # [128,66,66]
                for yb in range(NBLK):
                    y0 = yb * ROWBLK
                    ps = pp.tile([COUT, ROWBLK * W], F32)
                    first = True
                    for dy in range(KK):
                        for dx in range(KK):
                            t = dy * KK + dx
                            nc.tensor.matmul(
                                ps[:],
                                wt[:, t * COUT : (t + 1) * COUT],
                                xrf[:, y0 + dy : y0 + dy + ROWBLK, dx : dx + W],
                                start=first,
                                stop=(dy == KK - 1 and dx == KK - 1),
                            )
                            first = False
                    if lb == BPC - 1 and yb == NBLK - 1:
                        ot = op.tile([COUT, ROWBLK * W], F32, tag="otf")
                        # final block split by PARTITION (cout halves): copies
                        # run on vector+scalar in parallel and each output DMA
                        # only needs 64 descriptors, halving the exit chain.
                        ph = COUT // 2
                        for h_, deng in ((0, nc.sync), (1, nc.scalar)):
                            sl = slice(h_ * ph, (h_ + 1) * ph)
                            if h_ == 0:
                                nc.vector.tensor_copy(ot[sl, :], ps[sl, :])
                            else:
                                nc.scalar.copy(ot[sl, :], ps[sl, :])
                            deng.dma_start(
                                o_d[lb][sl, W * y0 : W * y0 + ROWBLK * W],
                                ot[sl, :],
                            )
                    elif lb == BPC - 1 and yb == NBLK - 2:
                        # block before the final one ships alone (its pair
                        # partner takes the exit-critical path)
                        ot = op.tile([COUT, ROWBLK * W], F32, tag="ot1")
                        nc.vector.tensor_copy(ot[:], ps[:])
                        ring[nout % 2].dma_start(
                            o_d[lb][:, W * y0 : W * y0 + ROWBLK * W], ot[:]
                        )
                        nout += 1
                    else:
                        # stage two blocks per SBUF tile; ship them as one
                        # 2-block DMA (fewer DMAs = less ring pacing overhead
                        # ahead of the exit-critical final DMA)
                        if yb % 2 == 0:
                            otp = op.tile([COUT, 2 * ROWBLK * W], F32, tag="otp")
                        nc.vector.tensor_copy(
                            otp[:, (yb % 2) * ROWBLK * W : (yb % 2 + 1) * ROWBLK * W],
                            ps[:],
                        )
                        if yb % 2 == 1:
                            ring[nout % 2].dma_start(
                                o_d[lb][:, W * (y0 - ROWBLK) : W * y0 + ROWBLK * W],
                                otp[:],
                            )
                            nout += 1
    nc.compile()
    return nc


def _get_nc():
    key = ("nc_v12", WARMN)
    if key not in _CACHE:
        _CACHE[key] = _build()
    return _CACHE[key]


def kernel(x, weights):
    """x: [16,128,64,64] f32; weights: [128,128,9] f32 -> [2048,64,64] f32."""
    global LAST_EXEC_NS
    x = np.asarray(x, dtype=np.float32)
    w = np.asarray(weights, dtype=np.float32)
    # [cout, cin, k] -> [cin, k, cout] so tap k is a contiguous lhsT slice
    wT = np.ascontiguousarray(w.transpose(1, 2, 0)).reshape(CIN, KK * KK * COUT)
    wT = wT.astype(np.float16)
    xpad = np.zeros((B, CIN, PH, PW), np.float16)
    xpad[:, :, 1 : H + 1, 1 : W + 1] = x.astype(np.float16)
    xpad = xpad.reshape(B, CIN, XLEN)

    nc = _get_nc()
    xr = xpad.reshape(NCORES, BPC, CIN, XLEN)
    in_maps = [{"x": np.ascontiguousarray(xr[c]), "w": wT} for c in range(NCORES)]

    res = bass_utils.run_bass_kernel_spmd(
        nc, in_maps, core_ids=list(range(NCORES)), trace=TRACE
    )
    LAST_EXEC_NS = res.exec_time_ns

    arr = np.stack([res.results[c]["o"] for c in range(NCORES)])  # [8, 2, 128, 4096]
    # out[cout*B + b] = conv[b, cout], with b = core*BPC + lb
    arr = arr.transpose(2, 0, 1, 3).reshape(COUT, B, H, W)
    return np.ascontiguousarray(arr.reshape(COUT * B, H, W))
